# revision 20
# baseline (speedup 1.0000x reference)
"""PathCausalSelfAttention on 8 trn2 cores.

Sharding: core c -> batch b=c//4, head-group hg=c%4 (4 heads each).
Per core: qkv projection (q pre-scaled by 1e-6 on host), scores
S = q~.k + g.g via packed K=128 matmul, P = exp(0.125*S) with causal
mask, y = P@v with fused denominator row, out-projection against the
core's 256-row slice of W_out. Host sums the 4 head-group partials
per batch.
"""

import numpy as np

import concourse.bacc as bacc
import concourse.mybir as mybir
import concourse.tile as tile
from concourse import masks
from concourse.bass_utils import run_bass_kernel_spmd

B, L, D, H = 2, 2048, 1024, 16
HD = 64
NCORES = 8
NH = 4          # heads per core
PC = NH * HD    # 256 projection cols per core
FP = mybir.dt.float32

LT = L // 128   # 16 L-tiles
DC = D // 128   # 8 contraction chunks
VW = 2 * (HD + 1)  # 130: V' cols per L-tile per pair (2 heads + ones cols)


def _emit(nc, tc):
    x_b = nc.declare_dram_parameter("x_b", [L, D], FP, isOutput=False)
    g_s = nc.declare_dram_parameter("g_s", [L, PC], FP, isOutput=False)
    wqkv = nc.declare_dram_parameter("wqkv", [D, 3 * PC], FP, isOutput=False)
    wout = nc.declare_dram_parameter("wout", [PC, D], FP, isOutput=False)
    out_p = nc.declare_dram_parameter("out_p", [L, D], FP, isOutput=True)

    perm1 = tc.alloc_tile_pool(name="perm1", bufs=1)
    ident = perm1.tile([128, 128], FP, name="ident")
    ut = perm1.tile([128, 128], FP, name="ut")
    ones_row = perm1.tile([1, HD], FP, name="ones_row")
    qpack = [perm1.tile([128, L], FP, name=f"qpack{h}") for h in range(NH)]
    kpack = [perm1.tile([128, L], FP, name=f"kpack{h}") for h in range(NH)]
    vp = [perm1.tile([128, LT * VW], FP, name=f"vp{p}") for p in range(2)]
    perm1.seal()

    masks.make_identity(nc, ident)
    masks.make_upper_triangular(nc, ut, val=1.0, diag=True)
    nc.gpsimd.memset(ones_row, 1.0)
    for p in range(2):
        nc.vector.memset(vp[p], 1.0)

    # ---- phase 1: transposes + projections ----
    with (
        tc.tile_pool(name="wqp", bufs=1) as wqpool,
        tc.tile_pool(name="xTp", bufs=1) as xTpool,
        tc.tile_pool(name="xin", bufs=4) as xpool,
        tc.tile_pool(name="gin", bufs=2) as gpool,
        tc.tile_pool(name="tp4", bufs=3, space="PSUM") as tp4pool,
        tc.tile_pool(name="tpg", bufs=2, space="PSUM") as tpgpool,
        tc.tile_pool(name="pj", bufs=3, space="PSUM") as pjpool,
    ):
        wq_sb = []
        for d in range(DC):
            w = wqpool.tile([128, 3 * PC], FP, name=f"wq{d}")
            nc.sync.dma_start(out=w, in_=wqkv[128 * d:128 * (d + 1), :])
            wq_sb.append(w)
        xT = [xTpool.tile([128, L], FP, name=f"xT{d}") for d in range(DC)]

        for ig in range(4):  # groups of 4 L-tiles
            xs = []
            for k in range(4):
                i = 4 * ig + k
                x_sb = xpool.tile([128, D], FP, name="xin")
                nc.sync.dma_start(out=x_sb, in_=x_b[128 * i:128 * (i + 1), :])
                xs.append(x_sb)
            for d in range(DC):
                tp = tp4pool.tile([128, 512], FP, name="tp4")
                for k in range(4):
                    nc.tensor.transpose(
                        tp[:, 128 * k:128 * (k + 1)],
                        xs[k][:, 128 * d:128 * (d + 1)], ident)
                nc.vector.tensor_copy(
                    xT[d][:, 512 * ig:512 * (ig + 1)], tp)

        for i in range(LT):
            g_sb = gpool.tile([128, PC], FP, name="gin")
            nc.sync.dma_start(out=g_sb, in_=g_s[128 * i:128 * (i + 1), :])
            for pr in range(2):
                tpg = tpgpool.tile([128, 128], FP, name="tpg")
                nc.tensor.transpose(tpg, g_sb[:, 128 * pr:128 * (pr + 1)], ident)
                for hh in range(2):
                    h = 2 * pr + hh
                    nc.vector.tensor_copy(
                        qpack[h][HD:128, 128 * i:128 * (i + 1)],
                        tpg[HD * hh:HD * (hh + 1), :])
        # kpack g-rows copied from qpack in one shot per head (on ACT)
        for h in range(NH):
            nc.scalar.copy(kpack[h][HD:128, :], qpack[h][HD:128, :])

        # q/k projections: out = Wcol.T @ xT  -> [128 (2 heads), 512] psum
        for qk in range(2):
            dest = qpack if qk == 0 else kpack
            for pr in range(2):
                base = PC * qk + 128 * pr
                for nch in range(4):
                    ps = pjpool.tile([128, 512], FP, name="pj")
                    for d in range(DC):
                        nc.tensor.matmul(
                            ps, lhsT=wq_sb[d][:, base:base + 128],
                            rhs=xT[d][:, 512 * nch:512 * (nch + 1)],
                            start=(d == 0), stop=(d == DC - 1))
                    for hh in range(2):
                        nc.vector.tensor_copy(
                            dest[2 * pr + hh][0:HD, 512 * nch:512 * (nch + 1)],
                            ps[HD * hh:HD * (hh + 1), :])

        # v in natural [L, cols] layout: lhsT = xT chunk, rhs = Wv chunk
        for i in range(LT):
            ps = pjpool.tile([128, PC], FP, name="pj", padded_shape=[128, 512])
            for d in range(DC):
                nc.tensor.matmul(
                    ps, lhsT=xT[d][:, 128 * i:128 * (i + 1)],
                    rhs=wq_sb[d][:, 2 * PC:3 * PC],
                    start=(d == 0), stop=(d == DC - 1))
            for pr in range(2):
                for hh in range(2):
                    nc.vector.tensor_copy(
                        vp[pr][:, VW * i + (HD + 1) * hh:
                               VW * i + (HD + 1) * hh + HD],
                        ps[:, 128 * pr + HD * hh:128 * pr + HD * (hh + 1)])

    # ---- phase 2: attention per head ----
    perm2 = tc.alloc_tile_pool(name="perm2", bufs=1)
    wout_sb = [perm2.tile([128, D], FP, name=f"wo{pr}") for pr in range(2)]
    ytsb = [perm2.tile([128, L], FP, name=f"ytsb{p}") for p in range(2)]
    bc_sb = perm2.tile([HD, L], FP, name="bc_sb")
    recip_sb = perm2.tile([1, L], FP, name="recip_sb")
    perm2.seal()
    for pr in range(2):
        nc.sync.dma_start(out=wout_sb[pr], in_=wout[128 * pr:128 * (pr + 1), :])

    with (
        tc.tile_pool(name="sc", bufs=2, space="PSUM") as scpool,
        tc.tile_pool(name="yT", bufs=1, space="PSUM") as ypool,
        tc.tile_pool(name="pt", bufs=2) as ptpool,
    ):
        for h in range(NH):
            pr, hh = h // 2, h % 2
            yT = ypool.tile([HD + 1, L], FP, name="yT")
            pts = {}
            for j in range(LT + 1):
                if j < LT:
                    a0 = 128 * j
                    ptj = ptpool.tile([128, L], FP, name="pt")
                    pts[j] = ptj
                    p0 = a0
                    while p0 < L:
                        p1 = min(L, (p0 // 1024 + 1) * 1024)
                        sc = scpool.tile([128, 1024], FP, name="sc")
                        b0 = p0
                        while b0 < p1:
                            b1 = min(p1, b0 + 512)
                            nc.tensor.matmul(
                                sc[:, b0 - p0:b1 - p0],
                                lhsT=kpack[h][:, a0:a0 + 128],
                                rhs=qpack[h][:, b0:b1],
                                start=True, stop=True)
                            b0 = b1
                        nc.scalar.activation(
                            ptj[:, p0:p1], sc[:, 0:p1 - p0],
                            mybir.ActivationFunctionType.Exp, scale=0.125)
                        p0 = p1
                    nc.vector.tensor_mul(
                        ptj[:, a0:a0 + 128], ptj[:, a0:a0 + 128], ut)
                if j > 0:
                    jj = j - 1
                    pv = pts.pop(jj)
                    for p in range(4):
                        q0 = max(128 * jj, 512 * p)
                        q1 = 512 * (p + 1)
                        if q0 >= q1:
                            continue
                        nc.tensor.matmul(
                            yT[:, q0:q1],
                            lhsT=vp[pr][:, VW * jj + (HD + 1) * hh:
                                        VW * jj + (HD + 1) * hh + HD + 1],
                            rhs=pv[:, q0:q1],
                            start=(jj == 0), stop=(jj == min(LT - 1, 4 * p + 3)))
            nc.vector.reciprocal(recip_sb, yT[HD:HD + 1, :])
            for c in range(2):
                bc = scpool.tile([128, 1024], FP, name="sc")
                for s in range(2):
                    nc.tensor.matmul(
                        bc[0:HD, 512 * s:512 * (s + 1)],
                        lhsT=ones_row,
                        rhs=recip_sb[:, 1024 * c + 512 * s:1024 * c + 512 * (s + 1)],
                        start=True, stop=True)
                nc.vector.tensor_copy(bc_sb[:, 1024 * c:1024 * (c + 1)], bc[0:HD, :])
            nc.vector.tensor_mul(
                ytsb[pr][HD * hh:HD * (hh + 1), :], yT[0:HD, :], bc_sb)

    # ---- phase 3: out projection ----
    with (
        tc.tile_pool(name="op", bufs=4, space="PSUM") as opool,
        tc.tile_pool(name="ob", bufs=4) as obpool,
    ):
        for lt in range(LT):
            for n2 in range(2):
                ops = opool.tile([128, 512], FP, name="op")
                for pr in range(2):
                    nc.tensor.matmul(
                        ops, lhsT=ytsb[pr][:, 128 * lt:128 * (lt + 1)],
                        rhs=wout_sb[pr][:, 512 * n2:512 * (n2 + 1)],
                        start=(pr == 0), stop=(pr == 1))
                ob = obpool.tile([128, 512], FP, name="ob")
                nc.scalar.copy(ob, ops)
                nc.sync.dma_start(
                    out=out_p[128 * lt:128 * (lt + 1), 512 * n2:512 * (n2 + 1)],
                    in_=ob)
    perm2.release()
    perm1.release()


_NC = None


def build_nc():
    global _NC
    if _NC is None:
        nc = bacc.Bacc("TRN2", target_bir_lowering=False)
        with tile.TileContext(nc) as tc:
            _emit(nc, tc)
        nc.finalize()
        _NC = nc
    return _NC


def prep_in_maps(x, g, W_qkv, W_out):
    x = np.ascontiguousarray(x, dtype=np.float32)
    g = np.ascontiguousarray(g, dtype=np.float32)
    W_qkv = np.asarray(W_qkv, dtype=np.float32)
    W_out = np.asarray(W_out, dtype=np.float32)
    in_maps = []
    for c in range(NCORES):
        b, hg = c // 4, c % 4
        lo = PC * hg
        wq = W_qkv[:, lo:lo + PC] * np.float32(1e-6)
        wk = W_qkv[:, D + lo:D + lo + PC]
        wv = W_qkv[:, 2 * D + lo:2 * D + lo + PC]
        in_maps.append({
            "x_b": x[b],
            "g_s": np.ascontiguousarray(g[b][:, lo:lo + PC]),
            "wqkv": np.ascontiguousarray(
                np.concatenate([wq, wk, wv], axis=1)),
            "wout": np.ascontiguousarray(W_out[lo:lo + PC, :]),
        })
    return in_maps


def gather(results):
    out = np.zeros((B, L, D), dtype=np.float32)
    for c in range(NCORES):
        out[c // 4] += results[c]["out_p"]
    return out


def kernel(x, g, W_qkv, W_out):
    nc = build_nc()
    in_maps = prep_in_maps(x, g, W_qkv, W_out)
    res = run_bass_kernel_spmd(nc, in_maps, list(range(NCORES)))
    return gather(res.results)


# revision 36
# speedup vs baseline: 2.2247x; 2.2247x over previous
"""PathCausalSelfAttention on 8 trn2 cores.

Sharding: core c -> batch b=c//4, head-group hg=c%4 (4 heads each).
Dtypes: projections + PV in bf16 (1-pass matmul), scores fused
q~.k + g.g in fp32r (1-pass, ~1.6e-4), out-projection fp32r.
x arrives bf16 and is transposed by DMA (xbar), g stays fp32 via PE
transposes. Softmax denominator via ones-column in V', reciprocal on
ACT, broadcast via K=1 matmul. Host sums 4 head-group partials/batch.
"""

import numpy as np
import ml_dtypes

import concourse.bacc as bacc
import concourse.mybir as mybir
import concourse.tile as tile
from concourse import masks
from concourse.bass_utils import run_bass_kernel_spmd

B, L, D, H = 2, 2048, 1024, 16
HD = 64
NCORES = 8
NH = 4          # heads per core
PC = NH * HD    # 256 projection cols per core
FP = mybir.dt.float32
FR = mybir.dt.float32r
BF = mybir.dt.bfloat16
AF = mybir.ActivationFunctionType

LT = L // 128   # 16 L-tiles
DC = D // 128   # 8 contraction chunks
VW = 2 * (HD + 1)  # 130: V' cols per L-tile per pair (2 heads + ones cols)


def _emit(nc, tc):
    x_bf = nc.declare_dram_parameter("x_bf", [L, D], BF, isOutput=False)
    g_s = nc.declare_dram_parameter("g_s", [L, PC], FP, isOutput=False)
    w_bf = nc.declare_dram_parameter("w_bf", [D, 3 * PC], BF, isOutput=False)
    wout = nc.declare_dram_parameter("wout", [PC, D], FR, isOutput=False)
    sel_d = nc.declare_dram_parameter("sel4", [NH, PC], FR, isOutput=False)
    out_p = nc.declare_dram_parameter("out_p", [L, D], FP, isOutput=True)

    perm1 = tc.alloc_tile_pool(name="perm1", bufs=1)
    ident = perm1.tile([128, 128], FP, name="ident")
    ut = perm1.tile([128, 128], BF, name="ut")
    qpack = [perm1.tile([128, L], FR, name=f"qpack{h}") for h in range(NH)]
    kpack = [perm1.tile([128, L], FR, name=f"kpack{h}") for h in range(NH)]
    vp = [perm1.tile([128, LT * VW], BF, name=f"vp{p}") for p in range(2)]
    perm1.seal()

    masks.make_identity(nc, ident)
    masks.make_upper_triangular(nc, ut, val=1.0, diag=True)
    for p in range(2):
        nc.vector.memset(vp[p], 1.0)

    # ---- phase 1: transposes + projections ----
    with (
        tc.tile_pool(name="wp", bufs=1) as wpool,
        tc.tile_pool(name="xTp", bufs=1) as xTpool,
        tc.tile_pool(name="gin", bufs=2) as gpool,
        tc.tile_pool(name="tpg", bufs=2, space="PSUM") as tpgpool,
        tc.tile_pool(name="pj", bufs=6, space="PSUM") as pjpool,
    ):
        w_sb = []
        for d in range(DC):
            w = wpool.tile([128, 3 * PC], BF, name=f"w{d}")
            nc.sync.dma_start(out=w, in_=w_bf[128 * d:128 * (d + 1), :])
            w_sb.append(w)
        xT = []
        for d in range(DC):
            t = xTpool.tile([128, L], BF, name=f"xT{d}")
            nc.sync.dma_start(out=t, in_=x_bf[:, 128 * d:128 * (d + 1)],
                              transpose=True)
            xT.append(t)

        # g: PE transpose (fp32) into the g-rows of qpack
        for i in range(LT):
            g_sb = gpool.tile([128, PC], FP, name="gin")
            nc.sync.dma_start(out=g_sb, in_=g_s[128 * i:128 * (i + 1), :])
            for pr in range(2):
                tpg = tpgpool.tile([128, 128], FP, name="tpg")
                nc.tensor.transpose(tpg, g_sb[:, 128 * pr:128 * (pr + 1)], ident)
                for hh in range(2):
                    h = 2 * pr + hh
                    nc.vector.tensor_copy(
                        qpack[h][HD:128, 128 * i:128 * (i + 1)],
                        tpg[HD * hh:HD * (hh + 1), :])
        for h in range(NH):
            nc.scalar.copy(kpack[h][HD:128, :], qpack[h][HD:128, :])

        # q/k projections (bf16): psum [128 (2 heads), 512]
        for qk in range(2):
            dest = qpack if qk == 0 else kpack
            for pr in range(2):
                base = PC * qk + 128 * pr
                for nch in range(4):
                    ps = pjpool.tile([128, 512], FP, name="pj")
                    for d in range(DC):
                        nc.tensor.matmul(
                            ps, lhsT=w_sb[d][:, base:base + 128],
                            rhs=xT[d][:, 512 * nch:512 * (nch + 1)],
                            start=(d == 0), stop=(d == DC - 1))
                    for hh in range(2):
                        nc.vector.tensor_copy(
                            dest[2 * pr + hh][0:HD, 512 * nch:512 * (nch + 1)],
                            ps[HD * hh:HD * (hh + 1), :])

        # v (bf16) in natural [L, cols] layout
        for i in range(LT):
            ps = pjpool.tile([128, PC], FP, name="pj", padded_shape=[128, 512])
            for d in range(DC):
                nc.tensor.matmul(
                    ps, lhsT=xT[d][:, 128 * i:128 * (i + 1)],
                    rhs=w_sb[d][:, 2 * PC:3 * PC],
                    start=(d == 0), stop=(d == DC - 1))
            for pr in range(2):
                for hh in range(2):
                    nc.vector.tensor_copy(
                        vp[pr][:, VW * i + (HD + 1) * hh:
                               VW * i + (HD + 1) * hh + HD],
                        ps[:, 128 * pr + HD * hh:128 * pr + HD * (hh + 1)])

    # ---- phase 2: attention per head ----
    perm2 = tc.alloc_tile_pool(name="perm2", bufs=1)
    wout_sb = [perm2.tile([128, D], FR, name=f"wo{pr}") for pr in range(2)]
    ytsb = [perm2.tile([128, L], FR, name=f"ytsb{p}") for p in range(2)]
    den1 = perm2.tile([1, NH * L], FP, name="den1")
    rc1 = perm2.tile([1, NH * L], FR, name="rc1")
    ones_row = perm2.tile([1, HD], FR, name="ones_row")
    perm2.seal()
    nc.sync.dma_start(out=ones_row, in_=sel_d[0:1, 0:HD])
    for pr in range(2):
        nc.sync.dma_start(out=wout_sb[pr], in_=wout[128 * pr:128 * (pr + 1), :])

    with (
        tc.tile_pool(name="sc", bufs=2, space="PSUM") as scpool,
        tc.tile_pool(name="yT", bufs=1, space="PSUM") as ypool,
        tc.tile_pool(name="pt", bufs=2) as ptpool,
        tc.tile_pool(name="bcs", bufs=2) as bcpool,
    ):
        for h in range(NH):
            pr, hh = h // 2, h % 2
            yTt = ypool.tile([HD + 1, L], FP, name="yT")
            pts = {}
            for j in range(LT + 1):
                if j < LT:
                    a0 = 128 * j
                    ptj = ptpool.tile([128, L], BF, name="pt")
                    pts[j] = ptj
                    c0 = a0
                    while c0 < L:
                        c1 = min(L, c0 + 1024)
                        sct = scpool.tile([128, 1024], FP, name="sc")
                        b0 = c0
                        while b0 < c1:
                            b1 = min(c1, b0 + 512)
                            nc.tensor.matmul(
                                sct[:, b0 - c0:b1 - c0],
                                lhsT=kpack[h][:, a0:a0 + 128],
                                rhs=qpack[h][:, b0:b1],
                                start=True, stop=True)
                            b0 = b1
                        nc.scalar.activation(
                            ptj[:, c0:c1], sct[:, 0:c1 - c0],
                            AF.Exp, scale=0.125)
                        c0 = c1
                    nc.vector.tensor_mul(
                        ptj[:, a0:a0 + 128], ptj[:, a0:a0 + 128], ut)
                if j > 0:
                    jj = j - 1
                    pv = pts.pop(jj)
                    for p in range(4):
                        q0 = max(128 * jj, 512 * p)
                        q1 = 512 * (p + 1)
                        if q0 >= q1:
                            continue
                        nc.tensor.matmul(
                            yTt[:, q0:q1],
                            lhsT=vp[pr][:, VW * jj + (HD + 1) * hh:
                                        VW * jj + (HD + 1) * hh + HD + 1],
                            rhs=pv[:, q0:q1],
                            start=(jj == 0), stop=(jj == min(LT - 1, 4 * p + 3)))
            # evict raw y + den row; divide later (batched reciprocal)
            nc.vector.tensor_copy(ytsb[pr][HD * hh:HD * (hh + 1), :],
                                  yTt[0:HD, :])
            nc.vector.tensor_copy(den1[0:1, h * L:(h + 1) * L],
                                  yTt[HD:HD + 1, :])

        nc.scalar.activation(den1, den1, AF.Ln)
        nc.scalar.activation(rc1, den1, AF.Exp, scale=-1.0)
        for h in range(NH):
            pr, hh = h // 2, h % 2
            bcs = bcpool.tile([128, L], FR, name="bcs")
            r0 = HD * hh
            for c in range(2):
                bc = scpool.tile([128, 1024], FP, name="sc")
                for s in range(2):
                    o0 = 1024 * c + 512 * s
                    nc.tensor.matmul(
                        bc[0:HD, 512 * s:512 * (s + 1)],
                        lhsT=ones_row,
                        rhs=rc1[0:1, h * L + o0:h * L + o0 + 512],
                        start=True, stop=True)
                nc.vector.tensor_copy(bcs[r0:r0 + HD, 1024 * c:1024 * (c + 1)],
                                      bc[0:HD, :])
            nc.vector.tensor_mul(ytsb[pr][r0:r0 + HD, :],
                                 ytsb[pr][r0:r0 + HD, :],
                                 bcs[r0:r0 + HD, :])

    # ---- phase 3: out projection (fp32r) ----
    with (
        tc.tile_pool(name="op", bufs=4, space="PSUM") as opool,
        tc.tile_pool(name="ob", bufs=4) as obpool,
    ):
        for lt in range(LT):
            for n2 in range(2):
                ops = opool.tile([128, 512], FP, name="op")
                for pr in range(2):
                    nc.tensor.matmul(
                        ops, lhsT=ytsb[pr][:, 128 * lt:128 * (lt + 1)],
                        rhs=wout_sb[pr][:, 512 * n2:512 * (n2 + 1)],
                        start=(pr == 0), stop=(pr == 1))
                ob = obpool.tile([128, 512], FP, name="ob")
                if n2 == 0:
                    nc.scalar.copy(ob, ops)
                else:
                    nc.vector.tensor_copy(ob, ops)
                nc.sync.dma_start(
                    out=out_p[128 * lt:128 * (lt + 1), 512 * n2:512 * (n2 + 1)],
                    in_=ob)
    perm2.release()
    perm1.release()


_NC = None


def build_nc():
    global _NC
    if _NC is None:
        nc = bacc.Bacc("TRN2", target_bir_lowering=False)
        with tile.TileContext(nc) as tc:
            _emit(nc, tc)
        nc.finalize()
        _NC = nc
    return _NC


def prep_in_maps(x, g, W_qkv, W_out):
    x = np.ascontiguousarray(x, dtype=np.float32)
    g = np.ascontiguousarray(g, dtype=np.float32)
    W_qkv = np.asarray(W_qkv, dtype=np.float32)
    W_out = np.asarray(W_out, dtype=np.float32)
    x16 = [np.ascontiguousarray(x[b]).astype(ml_dtypes.bfloat16)
           for b in range(B)]
    sel = np.zeros((NH, PC), dtype=np.float32)
    for h in range(NH):
        sel[h, HD * h:HD * (h + 1)] = 1.0
    in_maps = []
    for c in range(NCORES):
        b, hg = c // 4, c % 4
        lo = PC * hg
        wq = W_qkv[:, lo:lo + PC] * np.float32(1e-6)
        wk = W_qkv[:, D + lo:D + lo + PC]
        wv = W_qkv[:, 2 * D + lo:2 * D + lo + PC]
        in_maps.append({
            "x_bf": x16[b],
            "g_s": np.ascontiguousarray(g[b][:, lo:lo + PC]),
            "w_bf": np.ascontiguousarray(
                np.concatenate([wq, wk, wv], axis=1)).astype(
                    ml_dtypes.bfloat16),
            "wout": np.ascontiguousarray(W_out[lo:lo + PC, :]),
            "sel4": sel,
        })
    return in_maps


def gather(results):
    out = np.zeros((B, L, D), dtype=np.float32)
    for c in range(NCORES):
        out[c // 4] += results[c]["out_p"]
    return out


def kernel(x, g, W_qkv, W_out):
    nc = build_nc()
    in_maps = prep_in_maps(x, g, W_qkv, W_out)
    res = run_bass_kernel_spmd(nc, in_maps, list(range(NCORES)))
    return gather(res.results)
